# revision 5
# baseline (speedup 1.0000x reference)
"""Multi-head attention (B=4, L=2048, D=768, H=12) on 8 trn2 NeuronCores.

Sharding: core c handles (batch b = c//2, head-group hg = c%2 of 6 heads).
Device layout is fully transposed ([feature, seq]) so attention needs no
on-chip transposes:
  Q^T/K^T = W q^T            [dh, L]   (dh on partitions)
  S^T chunk = (K chunk) Q^T  [128k, q] via lhsT=K^T-chunk, rhs=Q^T
  P^T = exp(S^T/8)           (no max subtraction: S ~ N(0,1), exp is safe)
  O^T+rowsums = V̂^T P^T     via lhsT = [V | 1] (V natural layout + ones col)
  out^T = woS^T O_cat^T      after scaling by 1/rowsum (1/s via exp(-ln s))
Host: transposes/casts inputs per core, sums the two head-group partials per
batch, adds bo. Biases are zero and mask is all-ones in this problem's input
spec; a numpy fallback handles any other values.
"""

import sys
from contextlib import ExitStack

import numpy as np

for _p in ("/opt/trn_rl_repo",):
    if _p not in sys.path:
        sys.path.insert(0, _p)

import ml_dtypes

import concourse.bass as bass
import concourse.bacc as bacc
import concourse.mybir as mybir
import concourse.tile as tile
from concourse.bass_utils import run_bass_kernel_spmd

F32 = mybir.dt.float32
BF16 = mybir.dt.bfloat16
AF = mybir.ActivationFunctionType
ALU = mybir.AluOpType

D_MODEL = 768
N_HEAD = 12
B = 4
L = 2048
DH = 64
NCORES = 8
HPC = 6        # heads per core
NT = 3         # head-pair tiles per core (2 heads / 128-partition tile)
CK = 6         # 128-row chunks of D_MODEL
KC = 16        # 128-row chunks of L (key dim)
QH = 2         # q halves
QW = L // QH   # 1024
P = 128

_NC = None


def _build():
    nc = bacc.Bacc("TRN2")
    qT = nc.dram_tensor("qT", [D_MODEL, L], BF16, kind="ExternalInput")
    kT = nc.dram_tensor("kT", [D_MODEL, L], BF16, kind="ExternalInput")
    vT = nc.dram_tensor("vT", [D_MODEL, L], BF16, kind="ExternalInput")
    wqT = nc.dram_tensor("wqT", [D_MODEL, HPC * DH], BF16, kind="ExternalInput")
    wkT = nc.dram_tensor("wkT", [D_MODEL, HPC * DH], BF16, kind="ExternalInput")
    wvT = nc.dram_tensor("wvT", [D_MODEL, HPC * DH], BF16, kind="ExternalInput")
    woS = nc.dram_tensor("woS", [HPC * DH, D_MODEL], BF16, kind="ExternalInput")
    outT = nc.dram_tensor("outT", [D_MODEL, L], F32, kind="ExternalOutput")

    with tile.TileContext(nc) as tc, ExitStack() as ctx:
        wpool = ctx.enter_context(tc.tile_pool(name="wpool", bufs=1))
        qkv = ctx.enter_context(tc.tile_pool(name="qkv", bufs=1))
        stage = ctx.enter_context(tc.tile_pool(name="stage", bufs=13))
        ppool = ctx.enter_context(tc.tile_pool(name="ppool", bufs=3))
        bcpool = ctx.enter_context(tc.tile_pool(name="bcpool", bufs=2))
        opool = ctx.enter_context(tc.tile_pool(name="opool", bufs=2))
        small = ctx.enter_context(tc.tile_pool(name="small", bufs=2))
        psA = ctx.enter_context(tc.tile_pool(name="psA", bufs=2, space="PSUM"))
        psOp = ctx.enter_context(tc.tile_pool(name="psOp", bufs=2, space="PSUM"))

        # ---- weights to SBUF ----
        wq_sb = wpool.tile([P, CK, HPC * DH], BF16, name="wq_sb")
        nc.sync.dma_start(wq_sb, wqT[:, :].rearrange("(c p) m -> p c m", p=P))
        wk_sb = wpool.tile([P, CK, HPC * DH], BF16, name="wk_sb")
        nc.sync.dma_start(wk_sb, wkT[:, :].rearrange("(c p) m -> p c m", p=P))
        wv_sb = wpool.tile([P, CK, HPC * DH], BF16, name="wv_sb")
        nc.sync.dma_start(wv_sb, wvT[:, :].rearrange("(c p) m -> p c m", p=P))
        wo_sb = wpool.tile([P, NT, D_MODEL], BF16, name="wo_sb")
        nc.sync.dma_start(wo_sb, woS[:, :].rearrange("(t p) e -> p t e", p=P))
        ones_sb = wpool.tile([1, DH], F32, name="ones_sb")
        nc.vector.memset(ones_sb, 1.0)

        # ---- stage q/k/v (transposed, bf16) in 128-row chunks ----
        def load_chunks(src, nm):
            view = src[:, :].rearrange("(c p) l -> c p l", p=P)
            tiles = []
            for c in range(CK):
                st = stage.tile([P, L], BF16, tag="stage", name=f"st_{nm}{c}")
                nc.sync.dma_start(st, view[c])
                tiles.append(st)
            return tiles

        q_st = load_chunks(qT, "q")
        k_st = load_chunks(kT, "k")
        v_st = load_chunks(vT, "v")

        QT = qkv.tile([P, NT, L], BF16, name="QT")
        KT = qkv.tile([P, NT, L], BF16, name="KT")
        Vn = qkv.tile([P, KC, HPC, DH + 1], BF16, name="Vn")
        nc.vector.memset(Vn[:, :, :, DH:DH + 1], 1.0)

        # ---- Q^T / K^T projections:  dst[dh, l] = sum_D w[dh, D] x[D, l] ----
        for dst, st_tiles, w_sb in ((QT, q_st, wq_sb), (KT, k_st, wk_sb)):
            for t in range(NT):
                for qh in range(QH):
                    ps = psA.tile([P, QW], F32, tag="ps", name="ps_proj")
                    for c in range(CK):
                        for n in range(2):
                            nc.tensor.matmul(
                                ps[:, n * 512:(n + 1) * 512],
                                w_sb[:, c, t * P:(t + 1) * P],
                                st_tiles[c][:, qh * QW + n * 512: qh * QW + (n + 1) * 512],
                                start=(c == 0),
                                stop=(c == CK - 1),
                            )
                    nc.vector.tensor_copy(dst[:, t, qh * QW:(qh + 1) * QW], ps)

        # ---- V natural layout: Vn[k, h, d] = sum_D v[k, D] wv[h*64+d, D] ----
        for kc in range(KC):
            psV = psA.tile([P, HPC * DH], F32, tag="ps", name="psV")
            for c in range(CK):
                nc.tensor.matmul(
                    psV,
                    v_st[c][:, kc * P:(kc + 1) * P],
                    wv_sb[:, c, :],
                    start=(c == 0),
                    stop=(c == CK - 1),
                )
            nc.vector.tensor_copy(
                Vn[:, kc, :, 0:DH],
                psV.rearrange("p (h d) -> p h d", d=DH),
            )

        # ---- attention + output projection ----
        Ocat = [qkv.tile([P, L], BF16, name=f"Ocat{t}") for t in range(NT)]
        for qh in range(QH):
            for t in range(NT):
                psO = [
                    psOp.tile([DH + 1, QW], F32, tag="po", name=f"psO{hi}")
                    for hi in range(2)
                ]
                for kc in range(KC):
                    pTs = []
                    for hi in range(2):
                        off = hi * DH
                        psS = psA.tile([P, QW], F32, tag="ps", name="psS")
                        for n in range(2):
                            nc.tensor.matmul(
                                psS[:, n * 512:(n + 1) * 512],
                                KT[off:off + DH, t, kc * P:(kc + 1) * P],
                                QT[off:off + DH, t, qh * QW + n * 512: qh * QW + (n + 1) * 512],
                                start=True,
                                stop=True,
                            )
                        pT = ppool.tile([P, QW], BF16, tag="pt", name="pT")
                        nc.scalar.activation(pT, psS, AF.Exp, scale=0.125)
                        pTs.append(pT)
                    for hi in range(2):
                        for n in range(2):
                            nc.tensor.matmul(
                                psO[hi][:, n * 512:(n + 1) * 512],
                                Vn[:, kc, t * 2 + hi, :],
                                pTs[hi][:, n * 512:(n + 1) * 512],
                                start=(kc == 0),
                                stop=(kc == KC - 1),
                            )
                # normalize by 1/rowsum (row DH of psO) and write into Ocat
                for hi in range(2):
                    off = hi * DH
                    lns = small.tile([1, QW], F32, tag="lns", name="lns")
                    nc.scalar.activation(lns, psO[hi][DH:DH + 1, :], AF.Ln)
                    rinv = small.tile([1, QW], F32, tag="lns", name="rinv")
                    nc.scalar.activation(rinv, lns, AF.Exp, scale=-1.0)
                    psB = psA.tile([DH, QW], F32, tag="ps", name="psB")
                    for n in range(2):
                        nc.tensor.matmul(
                            psB[:, n * 512:(n + 1) * 512],
                            ones_sb[0:1, :],
                            rinv[0:1, n * 512:(n + 1) * 512],
                            start=True,
                            stop=True,
                        )
                    bc = bcpool.tile([DH, QW], F32, tag="bc", name="bc")
                    nc.vector.tensor_copy(bc, psB)
                    nc.vector.tensor_tensor(
                        Ocat[t][off:off + DH, qh * QW:(qh + 1) * QW],
                        psO[hi][0:DH, :],
                        bc,
                        ALU.mult,
                    )
            # output projection for this q-half
            out_view = outT[:, :].rearrange("(e p) l -> e p l", p=P)
            for et in range(CK):
                psE = psA.tile([P, QW], F32, tag="ps", name="psE")
                for t in range(NT):
                    for n in range(2):
                        nc.tensor.matmul(
                            psE[:, n * 512:(n + 1) * 512],
                            wo_sb[:, t, et * P:(et + 1) * P],
                            Ocat[t][:, qh * QW + n * 512: qh * QW + (n + 1) * 512],
                            start=(t == 0),
                            stop=(t == NT - 1),
                        )
                osb = opool.tile([P, QW], F32, tag="ot", name="osb")
                nc.vector.tensor_copy(osb, psE)
                nc.sync.dma_start(out_view[et][:, qh * QW:(qh + 1) * QW], osb)

    nc.compile()
    return nc


def _get_nc():
    global _NC
    if _NC is None:
        _NC = _build()
    return _NC


def bench(in_maps, iters=10):
    """Time steady-state execution of the compiled NEFF across the 8 cores.

    Replicates bass2jax.run_bass_via_pjrt's sharded jit, keeps inputs
    device-resident, creates donated zero outputs on-device, and reports
    per-call wall times (min ~ HW exec + dispatch overhead).
    """
    import time

    import jax
    import jax.numpy as jnp
    from jax.sharding import Mesh, NamedSharding, PartitionSpec
    from jax.experimental.shard_map import shard_map

    from concourse import bass2jax as b2j
    from concourse.bass2jax import _bass_exec_p, install_neuronx_cc_hook

    nc = _get_nc()
    install_neuronx_cc_hook()

    partition_name = nc.partition_id_tensor.name if nc.partition_id_tensor else None
    in_names, out_names, out_avals = [], [], []
    for alloc in nc.m.functions[0].allocations:
        if not isinstance(alloc, mybir.MemoryLocationSet):
            continue
        name = alloc.memorylocations[0].name
        if alloc.kind == "ExternalInput":
            if name != partition_name:
                in_names.append(name)
        elif alloc.kind == "ExternalOutput":
            out_names.append(name)
            out_avals.append(
                jax.core.ShapedArray(tuple(alloc.tensor_shape), mybir.dt.np(alloc.dtype))
            )
    n_params = len(in_names)
    n_outs = len(out_names)
    all_names = in_names + out_names
    if partition_name is not None:
        all_names = all_names + [partition_name]

    def _body(*args):
        operands = list(args)
        if partition_name is not None:
            operands.append(b2j.partition_id_tensor())
        outs = _bass_exec_p.bind(
            *operands,
            out_avals=tuple(out_avals),
            in_names=tuple(all_names),
            out_names=tuple(out_names),
            lowering_input_output_aliases=(),
            sim_require_finite=True,
            sim_require_nnan=True,
            nc=nc,
        )
        return tuple(outs)

    devices = jax.devices()[:NCORES]
    mesh = Mesh(np.asarray(devices), ("core",))
    donate = tuple(range(n_params, n_params + n_outs))
    sharded = jax.jit(
        shard_map(
            _body,
            mesh=mesh,
            in_specs=(PartitionSpec("core"),) * (n_params + n_outs),
            out_specs=(PartitionSpec("core"),) * n_outs,
            check_rep=False,
        ),
        donate_argnums=donate,
        keep_unused=True,
    )
    sh = NamedSharding(mesh, PartitionSpec("core"))
    dev_in = [
        jax.device_put(
            np.concatenate([np.asarray(m[nm]) for m in in_maps], axis=0), sh
        )
        for nm in in_names
    ]
    zero_fns = [
        jax.jit(
            lambda av=av: jnp.zeros((av.shape[0] * NCORES,) + av.shape[1:], av.dtype),
            out_shardings=sh,
        )
        for av in out_avals
    ]

    times = []
    outs = None
    for i in range(iters + 1):
        zs = [f() for f in zero_fns]
        jax.block_until_ready(zs)
        t0 = time.perf_counter()
        outs = sharded(*dev_in, *zs)
        jax.block_until_ready(outs)
        t1 = time.perf_counter()
        if i > 0:  # skip warmup
            times.append(t1 - t0)
    return times, outs


def _reference_fallback(q, k, v, mask, wq, bq, wk, bk, wv, bv, wo, bo):
    def split(t):
        b, l, d = t.shape
        return t.reshape(b, l, N_HEAD, d // N_HEAD).transpose(0, 2, 1, 3)

    qh = split(q @ wq.T + bq)
    kh = split(k @ wk.T + bk)
    vh = split(v @ wv.T + bv)
    score = np.einsum("bhqd,bhkd->bhqk", qh, kh) / np.sqrt(np.float32(DH))
    score = np.where(mask == 0, -np.inf, score)
    score = score - score.max(axis=-1, keepdims=True)
    e = np.exp(score)
    attn = e / e.sum(axis=-1, keepdims=True)
    attn = np.nan_to_num(attn, nan=0.0)
    out = np.einsum("bhqk,bhkd->bhqd", attn, vh)
    b_, h, l, dh = out.shape
    out = out.transpose(0, 2, 1, 3).reshape(b_, l, h * dh)
    return (out @ wo.T + bo).astype(np.float32)


def _run_spmd(in_maps, trace=False):
    nc = _get_nc()
    return run_bass_kernel_spmd(nc, in_maps, core_ids=list(range(NCORES)), trace=trace)


def kernel(q, k, v, mask, wq, bq, wk, bk, wv, bv, wo, bo, _trace=False):
    q = np.asarray(q, dtype=np.float32)
    k = np.asarray(k, dtype=np.float32)
    v = np.asarray(v, dtype=np.float32)
    mask = np.asarray(mask)
    wq = np.asarray(wq, dtype=np.float32)
    wk = np.asarray(wk, dtype=np.float32)
    wv = np.asarray(wv, dtype=np.float32)
    wo = np.asarray(wo, dtype=np.float32)
    bq = np.asarray(bq, dtype=np.float32)
    bk = np.asarray(bk, dtype=np.float32)
    bv = np.asarray(bv, dtype=np.float32)
    bo = np.asarray(bo, dtype=np.float32)

    # the device kernel specializes on all-ones mask and zero q/k/v biases
    # (this problem's input spec); anything else goes to the numpy fallback.
    if (mask == 0).any() or any(
        np.abs(x).max() > 0 for x in (bq, bk, bv)
    ):
        return _reference_fallback(q, k, v, mask, wq, bq, wk, bk, wv, bv, wo, bo)

    bf = ml_dtypes.bfloat16
    in_maps = []
    for c in range(NCORES):
        b, hg = c // 2, c % 2
        sl = slice(hg * HPC * DH, (hg + 1) * HPC * DH)
        in_maps.append({
            "qT": np.ascontiguousarray(q[b].T).astype(bf),
            "kT": np.ascontiguousarray(k[b].T).astype(bf),
            "vT": np.ascontiguousarray(v[b].T).astype(bf),
            "wqT": np.ascontiguousarray(wq[sl, :].T).astype(bf),
            "wkT": np.ascontiguousarray(wk[sl, :].T).astype(bf),
            "wvT": np.ascontiguousarray(wv[sl, :].T).astype(bf),
            "woS": np.ascontiguousarray(wo[:, sl].T).astype(bf),
        })

    res = _run_spmd(in_maps, trace=_trace)
    outs = res.results if hasattr(res, "results") else res
    out = np.empty((B, L, D_MODEL), dtype=np.float32)
    for b in range(B):
        acc = outs[2 * b]["outT"].astype(np.float32) + outs[2 * b + 1]["outT"].astype(np.float32)
        out[b] = acc.T + bo
    if _trace:
        return out, res
    return out
